# revision 5
# baseline (speedup 1.0000x reference)
"""Trainium2 Bass kernel for ExportableCostVolume (cross-correlation cost volume).

out[b, d, h, w] = mean_c left[b,c,h,w] * right[b,c,h,w-d]   (w >= d, else 0)
B=4, C=128, H=256, W=512, D=128.

v3 strategy (8 NeuronCores, data-parallel over H stripes):
  Per (b, h) image row, per 128-wide w-block j (stationary = left cols,
  pre-scaled by 1/C on the host so no on-device scaling is needed):
    G[wi, v] = sum_c L[c, w0+wi] * R[c, ustart-v]      (TensorE, fp32 PSUM;
               the moving operand reads the R window reversed, so the needed
               band is per-partition contiguous: out[d, w0+wi] = G[wi, 127-wi+d])
  The PSUM->SBUF copies (DVE/Act alternating, fp32->fp16 cast) write the
  staging tile Gsb in [wi, j, v, h] layout (h innermost, stride HB): the
  band values for (wi, j) -- all 128 d's x HB h's -- then form ONE
  contiguous 512-element (1 KiB) run per (partition, j):
      flat col = j*(256*HB) + (127-wi+d)*HB + h  ->  [j*1024+508-4wi, +512)
  so the band-extraction store to HBM needs only 512 descriptors of 1 KiB
  per batch instead of 2048 of 256 B.  Sub-512B DMA descriptors pay a 2x
  read-modify-write penalty at HBM (cost model: dma_elem_sz < 512 -> 2x),
  which is what made the v2 store (+49us) the bottleneck; 1 KiB runs store
  at line rate.  The host performs the final [B,D,H,W] permute + fp32
  upcast + zeroing of the w<d corner (the device never writes it).

  DMA issue-path assignment (HWDGE rings are strict FIFO with head-of-line
  blocking, so a dependency-carrying DMA queued ahead of input loads would
  serialize the pipeline):
    - BOTH input loads on the sync (SP) HWDGE ring and nothing else there:
      loads carry only buffer-reuse waits, so they prefetch POOL_BUFS
      batches ahead.
    - The band store on the scalar (Act) HWDGE ring: the scalar engine has
      just executed its own half of the copies when it reaches this
      dma_start, so its wait is nearly satisfied; and no loads queue
      behind it.
    - memset on gpsimd (Pool engine is otherwise idle).
"""
import sys

sys.path.insert(0, "/opt/trn_rl_repo")

import numpy as np

import concourse.bass as bass
import concourse.mybir as mybir
import concourse.tile as tile
from concourse import bacc
from concourse.bass_utils import run_bass_kernel_spmd

B, C, H, W, D = 4, 128, 256, 512, 128
NCORES = 8
HPC = H // NCORES   # h rows per core
HB = 4              # h rows per pipeline batch
NHB = HPC // HB     # batches per (b, core)
NJ = W // 128       # w-blocks per row

POOL_BUFS = 4
MEMSET_ENGINE = "gpsimd"   # engine for the j=0 zero-tail fill
DTYPE_MODE = "fp16"        # input dtype tag (read by the timing harnesses)
# ABLATE: drop pipeline pieces for HW bottleneck hunting (timing only —
# output is garbage for any non-empty value).
#   "loads"   = input DMAs only
#   "mm"      = loads + matmuls + copies (no store)
ABLATE = ""

GROW = 256 * HB       # flat cols per j-block in Gsb
GCOLS = NJ * GROW     # flat row length of Gsb

_nc_cache = {}


def _build_nc(iters: int = 1):
    f16 = mybir.dt.float16

    nc = bacc.Bacc()
    left_s = nc.declare_dram_parameter("left", [B, C, HPC, W], f16, isOutput=False)
    right_s = nc.declare_dram_parameter("right", [B, C, HPC, W], f16, isOutput=False)
    # band layout: out_bt[b, hb, wi, j, d*HB + h]
    out_s = nc.declare_dram_parameter("out", [B, NHB, 128, NJ, 128 * HB], f16, isOutput=True)

    nbatch = B * NHB * iters

    def batch_src(t):
        b, hb = divmod(t % (B * NHB), NHB)
        return b, hb

    with tile.TileContext(nc) as tc:
        with (
            tc.tile_pool(name="pool", bufs=POOL_BUFS) as pool,
            tc.tile_pool(name="ps", bufs=4, space="PSUM") as ps,
        ):
            for t in range(nbatch):
                b, hb = batch_src(t)
                h0 = hb * HB
                Ls = pool.tile([128, HB, W], f16, tag="Ls")
                Rs = pool.tile([128, HB, W], f16, tag="Rs")
                Gsb = pool.tile([128, NJ, 256, HB], f16, tag="Gsb")
                # both input loads on the sync HWDGE ring: nothing with
                # downstream dependencies ever queues there, so loads
                # prefetch POOL_BUFS batches ahead
                nc.sync.dma_start(Ls[:], left_s[b, :, h0:h0 + HB, :])
                nc.sync.dma_start(Rs[:], right_s[b, :, h0:h0 + HB, :])
                if ABLATE == "loads":
                    continue
                # zero fill for the w<d corner of block j=0 (read by the
                # band store; host discards it, but keep SBUF
                # deterministic/finite)
                getattr(nc, MEMSET_ENGINE).memset(Gsb[:, 0, 128:256, :], 0.0)
                for h in range(HB):
                    for j in range(NJ):
                        ncols = 128 if j == 0 else 256
                        g = ps.tile([128, ncols], mybir.dt.float32, tag=f"g{ncols}")
                        # moving operand: R cols (h, u), u descending from ustart
                        ustart = j * 128 + 127
                        rev = bass.AP(
                            Rs.tensor,
                            Rs.offset + h * W + ustart,
                            [[HB * W, 128], [-1, ncols]],
                        )
                        nc.tensor.matmul(
                            g[:], Ls[:, h, j * 128:(j + 1) * 128], rev,
                            start=True, stop=True,
                        )
                        # h-interleaved staging: Gsb[:, j, v, h] (stride HB).
                        # DVE's streaming SIMD collapses on non-unit-stride
                        # writes (50/50 DVE/Act split measured 177us vs Act's
                        # per-element pipeline which tolerates strides), so
                        # ALL copies go on the scalar (Act) engine.
                        gdst = Gsb[:, j, 0:ncols, h]
                        nc.scalar.copy(gdst, g[:])
                if ABLATE == "mm":
                    continue
                # band-extraction store fused into one DMA: per partition wi
                # and block j the band occupies ONE contiguous 512-element
                # run starting at flat col j*1024 + (127-wi)*HB:
                #   out_bt[wi, j, d*HB+h] = Gsb[wi, j*1024 + (127-wi+d)*HB + h]
                src = bass.AP(
                    Gsb.tensor,
                    Gsb.offset + (128 - 1) * HB,
                    [[GCOLS - HB, 128], [GROW, NJ], [1, 128 * HB]],
                )
                # issued on the scalar engine's HWDGE ring: the scalar
                # engine has just executed its own half of the copies, so
                # the wait is nearly satisfied when it reaches this
                # dma_start -- no load ring head-of-line risk
                nc.scalar.dma_start(out_s[b, hb], src)

            if ABLATE in ("loads", "mm"):
                # keep out_s alive with a single trailing write of
                # whatever tile the ablated pipeline last wrote
                keep = Ls[:] if ABLATE == "loads" else Gsb[:, :, 0:128, :]
                nc.sync.dma_start(out_s[0, 0], keep)

    nc.finalize()
    return nc


def kernel(left: np.ndarray, right: np.ndarray, _iters: int = 1) -> np.ndarray:
    key = (_iters,)
    if key not in _nc_cache:
        _nc_cache[key] = _build_nc(_iters)
    nc = _nc_cache[key]

    # fold the 1/C mean scaling into the left features (host-side, free)
    left = (np.asarray(left, dtype=np.float32) * (1.0 / C)).astype(np.float16)
    right = np.asarray(right, dtype=np.float32).astype(np.float16)
    in_maps = []
    for k in range(NCORES):
        sl = slice(k * HPC, (k + 1) * HPC)
        in_maps.append({
            "left": np.ascontiguousarray(left[:, :, sl, :]),
            "right": np.ascontiguousarray(right[:, :, sl, :]),
        })
    res = run_bass_kernel_spmd(nc, in_maps, list(range(NCORES)))

    # host-side reassembly: out[b, d, k*HPC + hb*HB + h, j*128 + wi]
    #   = out_bt[b, hb, wi, j, d*HB + h]
    out = np.empty((B, D, H, W), np.float32)
    for k in range(NCORES):
        arr = res.results[k]["out"]  # [B, NHB, 128, NJ, 128*HB] fp16
        arr = arr.reshape(B, NHB, 128, NJ, 128, HB)
        out[:, :, k * HPC:(k + 1) * HPC, :] = (
            arr.transpose(0, 4, 1, 5, 3, 2)
            .reshape(B, D, HPC, W)
            .astype(np.float32)
        )
    # zero the invalid w<d corner (only w-block j=0 can contain it; the
    # device writes stale/garbage there)
    d_idx = np.arange(D)[:, None]
    w_idx = np.arange(128)[None, :]
    valid = (w_idx >= d_idx)[None, :, None, :]
    out[:, :, :, :128] = np.where(valid, out[:, :, :, :128], 0.0)
    return out
